# revision 1
# baseline (speedup 1.0000x reference)
"""Trainium2 Bass kernel for nn_ConvOnTree (gnn_message_passing).

Computation (reference):
    selected = points[indices]                      # [N, 81, 3]
    dist     = sum((selected - selected[:, :1])**2, -1) + 1
    data     = concat(selected, dist[..., None])    # [N, 81, 4]
    out      = einsum('njc,cjl->nl', dw * data, weight) + bias

Strategy: data-parallel over N across 8 NeuronCores. Each core holds the full
points table in HBM and gathers neighbor coordinates with per-tile indirect
DMAs (128 rows per instruction, one descriptor per partition). Distances are
computed on DVE against the tile's own points (indices[:, 0] == arange, so the
"first neighbor" is the point itself). The einsum contracts (j, c) = 324 on PE
in three 108-row passes with dw folded into the weights on the host; bias is
added on ACT. Output tiles are written as [8, 128] and transposed on the host
during unsharding.
"""
import sys
import types

sys.path.insert(0, "/opt/trn_rl_repo")
sys.path.insert(0, "/root/.axon_site")

import numpy as np
import concourse.bass as bass
import concourse.mybir as mybir
import concourse.tile as tile
from concourse.vector_clock import ScopedClock
from concourse.bass_utils import run_bass_kernel_spmd
from concourse.masks import make_identity

fp32 = mybir.dt.float32
i32 = mybir.dt.int32

N = 500000
K = 81
OUT = 8
NCORES = 8
PPC = N // NCORES            # 62500 points per core
TILE = 128
NT = (PPC + TILE - 1) // TILE  # 489 tiles per core
PADPC = NT * TILE            # 62592 padded points per core


def _patched_drain_and_barrier(self, tick_clock, wait_clock):
    # This walrus build's CTRL_NO struct accepts too few sync waits for the
    # tile tail drain; spread the waits across preceding SP nops.
    nops = [self.nc.sync.nop() for _ in range(30)]
    drain_inst = self.nc.sync.drain()
    wait_clock.add_sem_waits(
        drain_inst.ins, ScopedClock({None: tick_clock.global_clock})
    )
    waits = list(drain_inst.ins.sync_info.on_wait) if drain_inst.ins.sync_info else []
    if len(waits) > 1:
        drain_inst.ins.sync_info.on_wait = waits[:1]
        for w, nop in zip(waits[1:], nops):
            si = nop.ins.sync_info
            if si is None:
                nop.ins.sync_info = mybir.SyncInfo(on_wait=[w], on_update=[])
            else:
                si.on_wait.append(w)
    self.nc.all_engine_barrier()
    popped = self.nc._tile_sem_poison_stack.pop()
    assert popped is self._sem_poison
    self.nc.clear_and_free_semaphores(list(self.sems.allocated().values()))
    self.nc.all_engine_barrier()


tile.TileContext._drain_and_barrier = _patched_drain_and_barrier


def _install_ntff_hook():
    # The image's antenv lacks axon_hooks; register the ctypes NTFF hook so
    # trace=True can report HW exec time. Harmless if tracing is never used.
    try:
        from trn_agent_boot.trn_boot import _ntff_profile_via_ctypes

        hook = _ntff_profile_via_ctypes("/opt/axon/libaxon_pjrt.so")
        mod = types.ModuleType("antenv.axon_hooks")
        mod.get_axon_ntff_profile_hook = lambda: hook
        import antenv  # noqa: F401

        sys.modules["antenv.axon_hooks"] = mod
    except Exception:
        pass


_install_ntff_hook()


MAX_WAITS = 1  # this walrus build encodes only one sync wait per instruction


def split_excess_waits(nc):
    """Move sync waits beyond MAX_WAITS onto same-engine InstNoOp carriers
    inserted immediately before the over-limit instruction."""
    n_split = 0
    for f in nc.m.functions:
        for b in f.blocks:
            new_insts = []
            for inst in b.instructions:
                si = inst.sync_info
                if si is not None and si.on_wait and len(si.on_wait) > MAX_WAITS:
                    waits = list(si.on_wait)
                    for k, w in enumerate(waits[MAX_WAITS:]):
                        nop = mybir.InstNoOp(
                            name=f"{inst.name}-wsplit{k}", ins=[], outs=[])
                        nop.engine = inst.engine
                        nop.sync_info = mybir.SyncInfo(on_wait=[w], on_update=[])
                        new_insts.append(nop)
                        n_split += 1
                    si.on_wait = waits[:MAX_WAITS]
                new_insts.append(inst)
            if len(new_insts) != len(b.instructions):
                b.instructions[:] = new_insts
    return n_split


def build_program():
    nc = bass.Bass("TRN2", target_bir_lowering=False, debug=False,
                   num_devices=NCORES)
    table = nc.dram_tensor("table", [N, 3], fp32, kind="ExternalInput")
    idx_in = nc.dram_tensor("idx", [NT * TILE, K], i32, kind="ExternalInput")
    pt_in = nc.dram_tensor("ptile", [NT * TILE, 3], fp32, kind="ExternalInput")
    w2_in = nc.dram_tensor("w2", [324, OUT], fp32, kind="ExternalInput")
    bias_in = nc.dram_tensor("bias", [OUT, 1], fp32, kind="ExternalInput")
    out_d = nc.dram_tensor("out", [NT * OUT, TILE], fp32, kind="ExternalOutput")

    with tile.TileContext(nc) as tc:
        with (
            tc.tile_pool(name="const", bufs=1) as cpool,
            tc.tile_pool(name="work", bufs=4) as wpool,
            tc.tile_pool(name="ps", bufs=2, space="PSUM") as ppool,
            tc.tile_pool(name="pso", bufs=2, space="PSUM") as opool,
        ):
            ident = cpool.tile([128, 128], fp32, name="ident")
            make_identity(nc, ident[:])
            w2_tiles = []
            for p in range(3):
                w2p = cpool.tile([108, OUT], fp32, name=f"w2_{p}")
                nc.sync.dma_start(w2p[:], w2_in[108 * p:108 * (p + 1), :])
                w2_tiles.append(w2p)
            bias_t = cpool.tile([OUT, 1], fp32, name="bias_t")
            nc.sync.dma_start(bias_t[:], bias_in[:])

            for iv in range(NT):
                idx_t = wpool.tile([TILE, K], i32, name="idx_t")
                nc.sync.dma_start(idx_t[:], idx_in[iv * TILE:(iv + 1) * TILE])
                p_t = wpool.tile([TILE, 3], fp32, name="p_t")
                nc.sync.dma_start(p_t[:], pt_in[iv * TILE:(iv + 1) * TILE])

                # gathered tile: [128 pts, 81 nbrs, 8] — [:, :, 0:3] = xyz,
                # [:, :, 3] = dist, [:, :, 4:7] = scratch (dx, dy, dz)
                g = wpool.tile([TILE, K, 8], fp32, name="g")
                # j = 0 is the self neighbor (indices[:, 0] == arange): its
                # coordinates are the tile's own points, already on chip.
                nc.vector.tensor_copy(out=g[:, 0, 0:3], in_=p_t[:, :])
                for j in range(1, K):
                    nc.gpsimd.indirect_dma_start(
                        out=g[:, j, 0:3],
                        out_offset=None,
                        in_=table[:],
                        in_offset=bass.IndirectOffsetOnAxis(
                            ap=idx_t[:, j:j + 1], axis=0),
                    )

                # dist = (q - p)^2 summed + 1, computed in scratch lanes
                for c in range(3):
                    nc.vector.tensor_scalar(
                        out=g[:, :, 4 + c:5 + c], in0=g[:, :, c:c + 1],
                        scalar1=p_t[:, c:c + 1], scalar2=None,
                        op0=mybir.AluOpType.subtract)
                for c in range(3):
                    nc.vector.scalar_tensor_tensor(
                        out=g[:, :, 4 + c:5 + c], in0=g[:, :, 4 + c:5 + c],
                        scalar=1.0, in1=g[:, :, 4 + c:5 + c],
                        op0=mybir.AluOpType.mult, op1=mybir.AluOpType.mult)
                nc.vector.tensor_tensor(
                    out=g[:, :, 3:4], in0=g[:, :, 4:5], in1=g[:, :, 5:6],
                    op=mybir.AluOpType.add)
                nc.vector.tensor_tensor(
                    out=g[:, :, 3:4], in0=g[:, :, 3:4], in1=g[:, :, 6:7],
                    op=mybir.AluOpType.add)
                nc.vector.tensor_scalar(
                    out=g[:, :, 3:4], in0=g[:, :, 3:4], scalar1=1.0,
                    scalar2=None, op0=mybir.AluOpType.add)

                # compact [128, 81, 4-of-8] -> contiguous [128, 324]
                gc = wpool.tile([TILE, K, 4], fp32, name="gc")
                nc.vector.tensor_copy(out=gc[:, :, :], in_=g[:, :, 0:4])

                # einsum: contract (j, c) = 324 in three 108-row passes
                psum_o = opool.tile([OUT, TILE], fp32, name="psum_o")
                for p in range(3):
                    tp = ppool.tile([108, TILE], fp32, name="tp")
                    nc.tensor.transpose(
                        out=tp[:],
                        in_=bass.AP(gc.tensor, 108 * p, [[K * 4, TILE], [1, 108]]),
                        identity=ident[:])
                    mov = wpool.tile([108, TILE], fp32, name="mov")
                    nc.scalar.copy(out=mov[:], in_=tp[:])
                    nc.tensor.matmul(
                        psum_o[:], w2_tiles[p][:], mov[:],
                        start=(p == 0), stop=(p == 2))

                o_t = wpool.tile([OUT, TILE], fp32, name="o_t")
                nc.scalar.add(out=o_t[:], in_=psum_o[:], add=bias_t[:, 0:1])
                nc.sync.dma_start(out_d[iv * OUT:(iv + 1) * OUT], o_t[:])

    split_excess_waits(nc)
    return nc


_CACHED_NC = None


def kernel(points, indices, dw, weight, bias):
    global _CACHED_NC
    points = np.ascontiguousarray(points, dtype=np.float32)
    indices = np.ascontiguousarray(indices)
    dw = np.asarray(dw, dtype=np.float32)
    weight = np.asarray(weight, dtype=np.float32)
    bias = np.asarray(bias, dtype=np.float32)

    # Fold dw into the weights: W2[(j*4 + c), l] = dw[j, c] * weight[c, j, l]
    w2 = (dw[:, :, None] * weight.transpose(1, 0, 2)).reshape(324, OUT)
    w2 = np.ascontiguousarray(w2, dtype=np.float32)
    bias_col = np.ascontiguousarray(bias.reshape(OUT, 1))

    idx32 = indices.astype(np.int32)
    in_maps = []
    for c in range(NCORES):
        lo, hi = c * PPC, (c + 1) * PPC
        idx_pad = np.zeros((PADPC, K), dtype=np.int32)
        idx_pad[:PPC] = idx32[lo:hi]
        pt_pad = np.zeros((PADPC, 3), dtype=np.float32)
        pt_pad[:PPC] = points[lo:hi]
        in_maps.append({
            "table": points,
            "idx": idx_pad,
            "ptile": pt_pad,
            "w2": w2,
            "bias": bias_col,
        })

    global _last_in_maps
    _last_in_maps = in_maps
    if _CACHED_NC is None:
        _CACHED_NC = build_program()
    res = run_bass_kernel_spmd(_CACHED_NC, in_maps, core_ids=list(range(NCORES)))

    out = np.empty((N, OUT), dtype=np.float32)
    for c in range(NCORES):
        o = res.results[c]["out"].reshape(NT, OUT, TILE)
        o = o.transpose(0, 2, 1).reshape(PADPC, OUT)
        out[c * PPC:(c + 1) * PPC] = o[:PPC]
    return out



# revision 10
# speedup vs baseline: 14.3276x; 14.3276x over previous
"""Trainium2 Bass kernel for nn_ConvOnTree (gnn_message_passing).

Computation (reference):
    selected = points[indices]                      # [N, 81, 3]
    dist     = sum((selected - selected[:, :1])**2, -1) + 1
    data     = concat(selected, dist[..., None])    # [N, 81, 4]
    out      = einsum('njc,cjl->nl', dw * data, weight) + bias

Strategy: data-parallel over N across 8 NeuronCores. Each core keeps a
replicated 4-float-per-point table [x, y, z, ||p||^2 + 0.5] in HBM.

The gather dominates; per-instruction SWDGE overhead (~1us) makes the
naive one-DMA-per-neighbor approach (~40k instructions) 20x too slow.
Instead each 128-point tile is gathered by THREE batched indirect DMAs
(3888+3888+2592 descriptors of 16B) into a flat partition-0 buffer, then
one direct 128-descriptor SBUF->SBUF DMA redistributes 1296B per point
to a [128, 324] tile. The DGE splits each batched gather into 16
per-DMA-engine blocks of B=ndesc/16 descriptors; every block's first
descriptor (except engine 0's) fetches its index from position +127, so
B is kept a multiple of 81 - the corrupted chunk is then always a
point's j=0 (self) row, which is rebuilt from a direct load since
indices[:, 0] == arange. Host-side index layout compensates for the
block-linearized, partition-fastest offset consumption order.

dist = (||q||^2+.5) + (||p||^2+.5) - 2 p.q needs only the dot product on
DVE; the gathered tile is a contiguous [128, 324] block which PE
transposes in three 108-row passes and contracts against dw-folded
weights; bias is added on ACT. Output tiles are written as [8, 128] and
transposed on the host during unsharding.
"""
import sys
import types

sys.path.insert(0, "/opt/trn_rl_repo")
sys.path.insert(0, "/root/.axon_site")

import numpy as np
import concourse.bass as bass
import concourse.mybir as mybir
import concourse.tile as tile
from concourse.vector_clock import ScopedClock
from concourse.bass_utils import run_bass_kernel_spmd
from concourse.masks import make_identity

fp32 = mybir.dt.float32
i32 = mybir.dt.int32

N = 500000
K = 81
OUT = 8
NCORES = 8
PPC = N // NCORES            # 62500 points per core
TILE = 128
NT = 496                     # padded tiles per core (496*128 = 63488)
PADPC = NT * TILE
FLAT = TILE * K * 4          # 41472 floats: one tile, flat in partition 0

# per-tile gather instructions: (start point, npoints, idx columns)
# ndesc = npoints*81 must keep ndesc/16 a multiple of 81 (glitch alignment)
# and ndesc <= ~4080 (per-engine descriptor ring).
GATHERS = ((0, 48, 31), (48, 48, 31), (96, 32, 21))
IDXCOLS = sum(g[2] for g in GATHERS)  # 83


def _patched_drain_and_barrier(self, tick_clock, wait_clock):
    # This walrus build's CTRL_NO struct accepts too few sync waits for the
    # tile tail drain; spread the waits across preceding SP nops.
    nops = [self.nc.sync.nop() for _ in range(30)]
    drain_inst = self.nc.sync.drain()
    wait_clock.add_sem_waits(
        drain_inst.ins, ScopedClock({None: tick_clock.global_clock})
    )
    waits = list(drain_inst.ins.sync_info.on_wait) if drain_inst.ins.sync_info else []
    if len(waits) > 1:
        drain_inst.ins.sync_info.on_wait = waits[:1]
        for w, nop in zip(waits[1:], nops):
            si = nop.ins.sync_info
            if si is None:
                nop.ins.sync_info = mybir.SyncInfo(on_wait=[w], on_update=[])
            else:
                si.on_wait.append(w)
    self.nc.all_engine_barrier()
    popped = self.nc._tile_sem_poison_stack.pop()
    assert popped is self._sem_poison
    self.nc.clear_and_free_semaphores(list(self.sems.allocated().values()))
    self.nc.all_engine_barrier()


tile.TileContext._drain_and_barrier = _patched_drain_and_barrier


def _install_ntff_hook():
    # The image's antenv lacks axon_hooks; register the ctypes NTFF hook so
    # trace=True can report HW exec time. Harmless if tracing is never used.
    try:
        from trn_agent_boot.trn_boot import _ntff_profile_via_ctypes

        hook = _ntff_profile_via_ctypes("/opt/axon/libaxon_pjrt.so")
        mod = types.ModuleType("antenv.axon_hooks")
        mod.get_axon_ntff_profile_hook = lambda: hook
        import antenv  # noqa: F401

        sys.modules["antenv.axon_hooks"] = mod
    except Exception:
        pass


_install_ntff_hook()


MAX_WAITS = 1  # this walrus build encodes only one sync wait per instruction


def split_excess_waits(nc):
    """Move sync waits beyond MAX_WAITS onto same-engine InstNoOp carriers
    inserted immediately before the over-limit instruction."""
    n_split = 0
    for f in nc.m.functions:
        for b in f.blocks:
            new_insts = []
            for inst in b.instructions:
                si = inst.sync_info
                if si is not None and si.on_wait and len(si.on_wait) > MAX_WAITS:
                    waits = list(si.on_wait)
                    for k, w in enumerate(waits[MAX_WAITS:]):
                        nop = mybir.InstNoOp(
                            name=f"{inst.name}-wsplit{k}", ins=[], outs=[])
                        nop.engine = inst.engine
                        nop.sync_info = mybir.SyncInfo(on_wait=[w], on_update=[])
                        new_insts.append(nop)
                        n_split += 1
                    si.on_wait = waits[:MAX_WAITS]
                new_insts.append(inst)
            if len(new_insts) != len(b.instructions):
                b.instructions[:] = new_insts
    return n_split


def build_program():
    nc = bass.Bass("TRN2", target_bir_lowering=False, debug=False,
                   num_devices=NCORES)
    table = nc.dram_tensor("table", [N, 4], fp32, kind="ExternalInput")
    idx_in = nc.dram_tensor("idx", [NT * TILE, IDXCOLS], i32,
                            kind="ExternalInput")
    pt_in = nc.dram_tensor("ptile", [NT * TILE, 4], fp32,
                           kind="ExternalInput")
    w2_in = nc.dram_tensor("w2", [324, OUT], fp32, kind="ExternalInput")
    bias_in = nc.dram_tensor("bias", [OUT, 1], fp32, kind="ExternalInput")
    out_d = nc.dram_tensor("out", [NT * OUT, TILE], fp32,
                           kind="ExternalOutput")

    with tile.TileContext(nc) as tc:
        with (
            tc.tile_pool(name="const", bufs=1) as cpool,
            tc.tile_pool(name="flat", bufs=1) as fpool,
            tc.tile_pool(name="work", bufs=3) as wpool,
            tc.tile_pool(name="ps", bufs=2, space="PSUM") as ppool,
            tc.tile_pool(name="pso", bufs=2, space="PSUM") as opool,
        ):
            ident = cpool.tile([128, 128], fp32, name="ident")
            make_identity(nc, ident[:])
            w2_tiles = []
            for p in range(3):
                w2p = cpool.tile([108, OUT], fp32, name=f"w2_{p}")
                nc.sync.dma_start(w2p[:], w2_in[108 * p:108 * (p + 1), :])
                w2_tiles.append(w2p)
            bias_t = cpool.tile([OUT, 1], fp32, name="bias_t")
            nc.sync.dma_start(bias_t[:], bias_in[:])

            g0 = fpool.tile([1, FLAT], fp32, name="g0")

            for iv in range(NT):
                idx_t = wpool.tile([TILE, IDXCOLS], i32, name="idx_t")
                nc.sync.dma_start(idx_t[:], idx_in[iv * TILE:(iv + 1) * TILE])
                p_t = wpool.tile([TILE, 4], fp32, name="p_t")
                nc.sync.dma_start(p_t[:], pt_in[iv * TILE:(iv + 1) * TILE])

                # three batched gathers into the flat partition-0 buffer
                c0 = 0
                for (pstart, npts, ncols) in GATHERS:
                    ng = npts * K
                    nc.gpsimd.indirect_dma_start(
                        out=bass.AP(g0.tensor, pstart * K * 4,
                                    [[FLAT, 1], [4, ng], [1, 4]]),
                        out_offset=None,
                        in_=table[:],
                        in_offset=bass.IndirectOffsetOnAxis(
                            ap=idx_t[:, c0:c0 + ncols], axis=0),
                    )
                    c0 += ncols

                # redistribute: 1296B per point -> [128, 81, 4]
                g = wpool.tile([TILE, K, 4], fp32, name="g")
                nc.sync.dma_start(
                    bass.AP(g.tensor, 0, [[K * 4, TILE], [1, K * 4]]),
                    bass.AP(g0.tensor, 0,
                            [[FLAT, 1], [K * 4, TILE], [1, K * 4]]),
                )
                # rebuild the (possibly glitch-corrupted) j=0 self rows
                nc.scalar.copy(out=g[:, 0, 0:4], in_=p_t[:, :])

                # dist = (|q|^2+.5) + (|p|^2+.5) - 2 p.q  into channel 3
                s = wpool.tile([TILE, K], fp32, name="s")
                nc.vector.tensor_scalar(
                    out=s[:, :], in0=g[:, :, 0:1],
                    scalar1=p_t[:, 0:1], scalar2=None,
                    op0=mybir.AluOpType.mult)
                for c in (1, 2):
                    nc.vector.scalar_tensor_tensor(
                        out=s[:, :], in0=g[:, :, c:c + 1],
                        scalar=p_t[:, c:c + 1], in1=s[:, :],
                        op0=mybir.AluOpType.mult, op1=mybir.AluOpType.add)
                nc.vector.scalar_tensor_tensor(
                    out=g[:, :, 3:4], in0=s[:, :], scalar=-2.0,
                    in1=g[:, :, 3:4],
                    op0=mybir.AluOpType.mult, op1=mybir.AluOpType.add)
                nc.vector.tensor_scalar(
                    out=g[:, :, 3:4], in0=g[:, :, 3:4],
                    scalar1=p_t[:, 3:4], scalar2=None,
                    op0=mybir.AluOpType.add)

                # einsum: contract (j, c) = 324 in three 108-row passes
                psum_o = opool.tile([OUT, TILE], fp32, name="psum_o")
                for p in range(3):
                    tp = ppool.tile([108, TILE], fp32, name="tp")
                    nc.tensor.transpose(
                        out=tp[:],
                        in_=bass.AP(g.tensor, 108 * p,
                                    [[K * 4, TILE], [1, 108]]),
                        identity=ident[:])
                    mov = wpool.tile([108, TILE], fp32, name="mov")
                    nc.scalar.copy(out=mov[:], in_=tp[:])
                    nc.tensor.matmul(
                        psum_o[:], w2_tiles[p][:], mov[:],
                        start=(p == 0), stop=(p == 2))

                o_t = wpool.tile([OUT, TILE], fp32, name="o_t")
                nc.scalar.add(out=o_t[:], in_=psum_o[:], add=bias_t[:, 0:1])
                nc.sync.dma_start(out_d[iv * OUT:(iv + 1) * OUT], o_t[:])

    split_excess_waits(nc)
    return nc


def _build_idx_layout(idx_pad):
    """[PADPC, 81] int32 -> [PADPC, IDXCOLS] gather-offset layout.

    Within each gather instruction, descriptor k (point k//81, neighbor
    k%81 of this instruction's point group) consumes the offset at
    partition-fastest position k, EXCEPT descriptors at block starts
    k = m*B (m>0, B = ndesc/16) which consume position k+127. Those
    fetch a duplicate row into the j=0 chunk of a point, rebuilt later
    on-chip, so their own index value is dropped and block-start
    positions are zero-filled.
    """
    T = idx_pad.reshape(NT, TILE, K)
    cols = np.zeros((NT, TILE, IDXCOLS), dtype=np.int32)
    c0 = 0
    for (pstart, npts, ncols) in GATHERS:
        ng = npts * K
        B = ng // 16
        v = T[:, pstart:pstart + npts, :].reshape(NT, ng)
        k = np.arange(ng)
        keep = (k == 0) | (k % B != 0)
        pos = np.zeros((NT, ncols * TILE), dtype=np.int32)
        pos[:, k[keep]] = v[:, keep]
        cols[:, :, c0:c0 + ncols] = (
            pos.reshape(NT, ncols, TILE).transpose(0, 2, 1))
        c0 += ncols
    return cols.reshape(NT * TILE, IDXCOLS)


_CACHED_NC = None


def kernel(points, indices, dw, weight, bias):
    global _CACHED_NC
    points = np.ascontiguousarray(points, dtype=np.float32)
    indices = np.ascontiguousarray(indices)
    dw = np.asarray(dw, dtype=np.float32)
    weight = np.asarray(weight, dtype=np.float32)
    bias = np.asarray(bias, dtype=np.float32)

    # Replicated gather table: [x, y, z, ||p||^2 + 0.5] per point, so that
    # dist = tab[q,3] + tab[p,3] - 2 p.q  ==  ||q - p||^2 + 1
    tab = np.empty((N, 4), dtype=np.float32)
    tab[:, 0:3] = points
    tab[:, 3] = (points * points).sum(axis=1) + 0.5

    # Fold dw into the weights: W2[(j*4 + c), l] = dw[j, c] * weight[c, j, l]
    w2 = (dw[:, :, None] * weight.transpose(1, 0, 2)).reshape(324, OUT)
    w2 = np.ascontiguousarray(w2, dtype=np.float32)
    bias_col = np.ascontiguousarray(bias.reshape(OUT, 1))

    idx32 = indices.astype(np.int32)
    in_maps = []
    for c in range(NCORES):
        lo, hi = c * PPC, (c + 1) * PPC
        idx_pad = np.zeros((PADPC, K), dtype=np.int32)
        idx_pad[:PPC] = idx32[lo:hi]
        pt_pad = np.zeros((PADPC, 4), dtype=np.float32)
        pt_pad[:PPC] = tab[lo:hi]
        in_maps.append({
            "table": tab,
            "idx": _build_idx_layout(idx_pad),
            "ptile": pt_pad,
            "w2": w2,
            "bias": bias_col,
        })

    global _last_in_maps
    _last_in_maps = in_maps
    if _CACHED_NC is None:
        _CACHED_NC = build_program()
    res = run_bass_kernel_spmd(_CACHED_NC, in_maps, core_ids=list(range(NCORES)))

    out = np.empty((N, OUT), dtype=np.float32)
    for c in range(NCORES):
        o = res.results[c]["out"].reshape(NT, OUT, TILE)
        o = o.transpose(0, 2, 1).reshape(PADPC, OUT)
        out[c * PPC:(c + 1) * PPC] = o[:PPC]
    return out
